# revision 16
# baseline (speedup 1.0000x reference)
"""Linear attention ("Transformers are RNNs") on 8 Trainium2 NeuronCores.

Problem: N=8, L=S=8192, H=8, D=Dv=32, f32.
    phi(x) = elu(x)+1
    A[d,v] = sum_s phi(K)[s,d] V[s,v]        (the /v_length ... *v_length cancels exactly)
    b[d]   = sum_s phi(K)[s,d]
    out[l,v] = (sum_d phi(Q)[l,d] A[d,v]) / (sum_d phi(Q)[l,d] b[d] + EPS)

Sharding: batch element n -> core n (fully independent, no collectives).

Device design (v4):
  - bf16 compute throughout (rel err ~4e-3, gate is 2e-2): inputs are
    cast to bf16 on the host, halving DMA traffic; PSUM accumulation and
    the denominator/normalization stay f32.
  - Q pre-transposed on host to [H*D, L]: contraction dim d on SBUF
    partitions, contiguous DMA, no on-device transposes.
  - phi(x) = min(exp(x), 1 + relu(x))  (exactly elu(x)+1):
    e = Exp(x) (ScalarE); t = (x max 0)+1 (VectorE dual-op tensor_scalar,
    4x mode); phi = min(e, t) (VectorE tt, 2x mode).
  - Phase 1 (64 s-subtiles of 128): per 4-head group g one bf16 matmul
        lhsT = phi(K)[:, g]  [s=128, (j,d)=128]
        rhs  = [V_g | ones]  [s=128, 129]
    accumulated in PSUM[128,129] per group.  Diagonal 32x32 j-blocks of
    cols 0:128 are A_h; col 128 is b_h.
  - Phase 1.5: assemble per group: block-diag A [128,128] bf16 and
    block-diag b columns [128,4] bf16.
  - Phase 2 (64 l-subtiles): per group two matmuls sharing the same
    stationary phiQ^T slice: numer (N=128, lands directly in output
    layout) and den (N=4, batched per 8-subtile macro in one PSUM bank so
    the eps-add + reciprocal amortize).  Normalize with one broadcast
    tensor_tensor multiply per 2 subtiles; bf16 out, host casts to f32.
"""

import sys

for _p in ("/opt/trn_rl_repo",):
    if _p not in sys.path:
        sys.path.insert(0, _p)

import ml_dtypes
import numpy as np

from concourse import bacc, bass, mybir, tile
from concourse.bass_utils import run_bass_kernel_spmd

# ---------------------------------------------------------------- constants
N_BATCH = 8
L = 8192
S = 8192
H = 8
D = 32
HD = H * D  # 256
P = 128
EPS = 1e-6

F32 = mybir.dt.float32
BF16 = mybir.dt.bfloat16
AF = mybir.ActivationFunctionType
OP = mybir.AluOpType

MACRO = 8  # 128-row s-subtiles per phase-1 macro tile
N_MACRO = S // (P * MACRO)  # 8
QMACRO = 8  # l-subtiles per phase-2 macro
N_QMACRO = L // (P * QMACRO)  # 8

G = 2  # head groups (4 heads each)
VA = P + 1  # 129: V group columns + ones column
VR = G * VA  # 258: host-side V row: [V_g0 | 1 | V_g1 | 1]


def _bcast_last(ap, n):
    """Append a stride-0 dim of size n to an AP (free-dim broadcast)."""
    ap = ap.unsqueeze(ap.ndim)
    return ap.broadcast_to(tuple(ap.shape[:-1]) + (n,))


def _phi(nc, pool, x, fd):
    """phi(x) = elu(x)+1 = min(exp(x), 1 + relu(x)); x is [P, fd] bf16 SBUF."""
    e = pool.tile([P, fd], BF16, tag="phi_e")
    t = pool.tile([P, fd], BF16, tag="phi_t")
    phi = pool.tile([P, fd], BF16, tag="phi_o")
    nc.scalar.activation(e[:], x[:], AF.Exp)
    nc.vector.tensor_scalar(t[:], x[:], 0.0, 1.0, OP.max, OP.add)
    nc.vector.tensor_tensor(phi[:], e[:], t[:], OP.min)
    return phi


def _phi2(nc, pool, x, fd):
    """phi = (exp(x) min 1) + relu(x); exp and relu on ScalarE, one DVE
    scalar_tensor_tensor combines them (rebalances DVE -> ACT)."""
    e = pool.tile([P, fd], BF16, tag="phi_e")
    r = pool.tile([P, fd], BF16, tag="phi_r")
    phi = pool.tile([P, fd], BF16, tag="phi_o")
    nc.scalar.activation(e[:], x[:], AF.Exp)
    nc.scalar.activation(r[:], x[:], AF.Relu)
    nc.vector.scalar_tensor_tensor(phi[:], e[:], 1.0, r[:], OP.min, OP.add)
    return phi


def _build_body(nc, tc, qt, kk, vv, out):
    with (
        tc.tile_pool(name="io", bufs=4) as io,
        tc.tile_pool(name="ew", bufs=3) as ew,
        tc.tile_pool(name="misc", bufs=1) as misc,
        tc.tile_pool(name="small", bufs=3) as small,
        tc.tile_pool(name="outp", bufs=4) as outp,
    ):
        # ---------------- phase 1: A/b accumulation over S ----------------
        with tc.tile_pool(name="ps1", bufs=1, space="PSUM") as ps1:
            pacc = [
                ps1.tile([P, VA], F32, tag=f"pacc{g}", name=f"pacc{g}")
                for g in range(G)
            ]

            # HAM warm-up: a dense dummy matmul burst while the initial DMAs
            # prefill.  ~16 N=512 matmuls = ~5us of continuous PE activity
            # flips the clock gate to 8/8 (2.4 GHz); the real MM stream then
            # never idles long enough (>3.4us) to re-throttle.
            wz = misc.tile([P, 512], BF16, tag="warm", name="warm")
            nc.vector.memset(wz[:], 0.0)
            junk = ps1.tile([P, 512], F32, tag="junk", name="junk")
            for _ in range(16):
                nc.tensor.matmul(
                    junk[:], wz[:, 0:P], wz[:], start=True, stop=True
                )

            for m in range(N_MACRO):
                k_t = io.tile([P, MACRO * HD], BF16, tag="k_t")
                rows = slice(m * MACRO * P, (m + 1) * MACRO * P)
                nc.sync.dma_start(
                    k_t[:].rearrange("p (b c) -> p b c", b=MACRO),
                    kk[rows, :].rearrange("(b p) c -> p b c", p=P),
                )
                v_t = io.tile([P, MACRO * VR], BF16, tag="v_t")
                nc.sync.dma_start(
                    v_t[:].rearrange("p (b c) -> p b c", b=MACRO),
                    vv[rows, :].rearrange("(b p) c -> p b c", p=P),
                )

                phi = _phi(nc, ew, k_t, MACRO * HD)

                first = m == 0
                last = m == N_MACRO - 1
                for b in range(MACRO):
                    for g in range(G):
                        nc.tensor.matmul(
                            pacc[g][:],
                            phi[:, b * HD + g * P : b * HD + (g + 1) * P],
                            v_t[:, b * VR + g * VA : b * VR + (g + 1) * VA],
                            start=(first and b == 0),
                            stop=(last and b == MACRO - 1),
                        )

            # ------------- phase 1.5: block-diag A, block-diag b ----------
            amat = []
            bmat = []
            for g in range(G):
                ag = misc.tile([P, P], BF16, tag=f"amat{g}", name=f"amat{g}")
                bg = misc.tile([P, 4], BF16, tag=f"bmat{g}", name=f"bmat{g}")
                nc.vector.memset(ag[:], 0.0)
                nc.vector.memset(bg[:], 0.0)
                for j in range(4):
                    r0 = 32 * j
                    nc.scalar.copy(
                        ag[r0 : r0 + 32, r0 : r0 + 32],
                        pacc[g][r0 : r0 + 32, r0 : r0 + 32],
                    )
                    nc.scalar.copy(
                        bg[r0 : r0 + 32, j : j + 1],
                        pacc[g][r0 : r0 + 32, P : P + 1],
                    )
                amat.append(ag)
                bmat.append(bg)

            # keep PE warm across the phase-1.5 transition
            for _ in range(6):
                nc.tensor.matmul(
                    junk[:], wz[:, 0:P], wz[:], start=True, stop=True
                )

        # ---------------- phase 2: queries ----------------
        with (
            tc.tile_pool(name="ps2n", bufs=5, space="PSUM") as ps2n,
            tc.tile_pool(name="ps2d", bufs=2, space="PSUM") as ps2d,
        ):
            for mq in range(N_QMACRO):
                c0 = mq * QMACRO * P
                phis = []
                for g in range(G):
                    qt_t = io.tile(
                        [P, QMACRO * P], BF16, tag=f"qt{g}", name=f"qt{g}"
                    )
                    nc.sync.dma_start(
                        qt_t[:], qt[g * P : (g + 1) * P, c0 : c0 + QMACRO * P]
                    )
                    phis.append(_phi2(nc, ew, qt_t, QMACRO * P))

                # den PSUM for the whole macro: cols (sub, g, j)
                den_ps = ps2d.tile([P, QMACRO * G * 4], F32, tag="den_ps")
                numers = []
                nm = None
                for i in range(QMACRO):
                    if i % 2 == 0:
                        nm = ps2n.tile([P, 2 * HD], F32, tag="nm")
                        numers.append(nm)
                    for g in range(G):
                        w = phis[g][:, i * P : (i + 1) * P]
                        nc.tensor.matmul(
                            nm[:, (i % 2) * HD + g * P : (i % 2) * HD + (g + 1) * P],
                            w,
                            amat[g][:],
                            start=True,
                            stop=True,
                        )
                        nc.tensor.matmul(
                            den_ps[:, (i * G + g) * 4 : (i * G + g + 1) * 4],
                            w,
                            bmat[g][:],
                            start=True,
                            stop=True,
                        )

                den_sb = small.tile([P, QMACRO * G * 4], F32, tag="den_sb")
                rcp = small.tile([P, QMACRO * G * 4], F32, tag="rcp")
                nc.vector.tensor_scalar(den_sb[:], den_ps[:], EPS, None, OP.add)
                nc.vector.reciprocal(rcp[:], den_sb[:])

                for pr in range(QMACRO // 2):
                    out_t = outp.tile([P, 2 * HD], BF16, tag="out_t")
                    rv = rcp[:, 2 * pr * G * 4 : (2 * pr + 2) * G * 4].rearrange(
                        "p (s g j) -> p s g j", s=2, g=G
                    )
                    nc.vector.tensor_tensor(
                        out_t[:].rearrange(
                            "p (s g j c) -> p s g j c", s=2, g=G, c=32
                        ),
                        numers[pr][:].rearrange(
                            "p (s g j c) -> p s g j c", s=2, g=G, c=32
                        ),
                        _bcast_last(rv, 32),
                        OP.mult,
                    )
                    r0 = c0 + 2 * pr * P
                    nc.sync.dma_start(
                        out[r0 : r0 + 2 * P, :].rearrange("(s p) c -> p s c", p=P),
                        out_t[:].rearrange("p (s c) -> p s c", s=2),
                    )


_NC_CACHE = None


def build_nc():
    global _NC_CACHE
    if _NC_CACHE is not None:
        return _NC_CACHE
    nc = bacc.Bacc(
        "TRN2",
        target_bir_lowering=False,
        debug=False,
        enable_asserts=False,
        num_devices=N_BATCH,
    )
    qt = nc.dram_tensor("qt", [HD, L], BF16, kind="ExternalInput").ap()
    kk = nc.dram_tensor("kk", [S, HD], BF16, kind="ExternalInput").ap()
    vv = nc.dram_tensor("vv", [S, VR], BF16, kind="ExternalInput").ap()
    out = nc.dram_tensor("out", [L, HD], BF16, kind="ExternalOutput").ap()
    with tile.TileContext(nc) as tc:
        _build_body(nc, tc, qt, kk, vv, out)
    nc.compile()
    return nc


def make_in_maps(queries, keys, values):
    queries = np.asarray(queries, dtype=np.float32)
    keys = np.asarray(keys, dtype=np.float32)
    values = np.asarray(values, dtype=np.float32)
    bf = ml_dtypes.bfloat16
    in_maps = []
    for n in range(N_BATCH):
        v2 = values[n].reshape(S, HD)
        vva = np.ones((S, VR), dtype=bf)
        vva[:, 0:P] = v2[:, 0:P].astype(bf)
        vva[:, VA : VA + P] = v2[:, P : 2 * P].astype(bf)
        qt = np.ascontiguousarray(
            queries[n].transpose(1, 2, 0).reshape(HD, L).astype(bf)
        )  # [h*32+d, l]
        in_maps.append(
            {
                "qt": qt,
                "kk": np.ascontiguousarray(keys[n].reshape(S, HD).astype(bf)),
                "vv": vva,
            }
        )
    return in_maps


def run(queries, keys, values, trace=False, **kwargs):
    nc = build_nc()
    in_maps = make_in_maps(queries, keys, values)
    res = run_bass_kernel_spmd(
        nc, in_maps, core_ids=list(range(N_BATCH)), trace=trace, **kwargs
    )
    outs = [
        res.results[n]["out"].astype(np.float32).reshape(L, H, D)
        for n in range(N_BATCH)
    ]
    return np.stack(outs, axis=0), res


def kernel(queries, keys, values):
    out, _ = run(queries, keys, values, trace=False)
    return out


# revision 17
# speedup vs baseline: 1.0033x; 1.0033x over previous
"""Linear attention ("Transformers are RNNs") on 8 Trainium2 NeuronCores.

Problem: N=8, L=S=8192, H=8, D=Dv=32, f32.
    phi(x) = elu(x)+1
    A[d,v] = sum_s phi(K)[s,d] V[s,v]        (the /v_length ... *v_length cancels exactly)
    b[d]   = sum_s phi(K)[s,d]
    out[l,v] = (sum_d phi(Q)[l,d] A[d,v]) / (sum_d phi(Q)[l,d] b[d] + EPS)

Sharding: batch element n -> core n (fully independent, no collectives).

Device design (v4):
  - bf16 compute throughout (rel err ~4e-3, gate is 2e-2): inputs are
    cast to bf16 on the host, halving DMA traffic; PSUM accumulation and
    the denominator/normalization stay f32.
  - Q pre-transposed on host to [H*D, L]: contraction dim d on SBUF
    partitions, contiguous DMA, no on-device transposes.
  - phi(x) = min(exp(x), 1 + relu(x))  (exactly elu(x)+1):
    e = Exp(x) (ScalarE); t = (x max 0)+1 (VectorE dual-op tensor_scalar,
    4x mode); phi = min(e, t) (VectorE tt, 2x mode).
  - Phase 1 (64 s-subtiles of 128): per 4-head group g one bf16 matmul
        lhsT = phi(K)[:, g]  [s=128, (j,d)=128]
        rhs  = [V_g | ones]  [s=128, 129]
    accumulated in PSUM[128,129] per group.  Diagonal 32x32 j-blocks of
    cols 0:128 are A_h; col 128 is b_h.
  - Phase 1.5: assemble per group: block-diag A [128,128] bf16 and
    block-diag b columns [128,4] bf16.
  - Phase 2 (64 l-subtiles): per group two matmuls sharing the same
    stationary phiQ^T slice: numer (N=128, lands directly in output
    layout) and den (N=4, batched per 8-subtile macro in one PSUM bank so
    the eps-add + reciprocal amortize).  Normalize with one broadcast
    tensor_tensor multiply per 2 subtiles; bf16 out, host casts to f32.
"""

import sys

for _p in ("/opt/trn_rl_repo",):
    if _p not in sys.path:
        sys.path.insert(0, _p)

import ml_dtypes
import numpy as np

from concourse import bacc, bass, mybir, tile
from concourse.bass_utils import run_bass_kernel_spmd

# ---------------------------------------------------------------- constants
N_BATCH = 8
L = 8192
S = 8192
H = 8
D = 32
HD = H * D  # 256
P = 128
EPS = 1e-6

F32 = mybir.dt.float32
BF16 = mybir.dt.bfloat16
AF = mybir.ActivationFunctionType
OP = mybir.AluOpType

MACRO = 8  # 128-row s-subtiles per phase-1 macro tile
N_MACRO = S // (P * MACRO)  # 8
QMACRO = 8  # l-subtiles per phase-2 macro
N_QMACRO = L // (P * QMACRO)  # 8

G = 2  # head groups (4 heads each)
VA = P + 1  # 129: V group columns + ones column
VR = G * VA  # 258: host-side V row: [V_g0 | 1 | V_g1 | 1]


def _bcast_last(ap, n):
    """Append a stride-0 dim of size n to an AP (free-dim broadcast)."""
    ap = ap.unsqueeze(ap.ndim)
    return ap.broadcast_to(tuple(ap.shape[:-1]) + (n,))


def _phi(nc, pool, x, fd):
    """phi(x) = elu(x)+1 = min(exp(x), 1 + relu(x)); x is [P, fd] bf16 SBUF."""
    e = pool.tile([P, fd], BF16, tag="phi_e")
    t = pool.tile([P, fd], BF16, tag="phi_t")
    phi = pool.tile([P, fd], BF16, tag="phi_o")
    nc.scalar.activation(e[:], x[:], AF.Exp)
    nc.vector.tensor_scalar(t[:], x[:], 0.0, 1.0, OP.max, OP.add)
    nc.vector.tensor_tensor(phi[:], e[:], t[:], OP.min)
    return phi


def _phi2(nc, pool, x, fd):
    """phi = (exp(x) min 1) + relu(x); exp and relu on ScalarE, one DVE
    scalar_tensor_tensor combines them (rebalances DVE -> ACT)."""
    e = pool.tile([P, fd], BF16, tag="phi_e")
    r = pool.tile([P, fd], BF16, tag="phi_r")
    phi = pool.tile([P, fd], BF16, tag="phi_o")
    nc.scalar.activation(e[:], x[:], AF.Exp)
    nc.scalar.activation(r[:], x[:], AF.Relu)
    nc.vector.scalar_tensor_tensor(phi[:], e[:], 1.0, r[:], OP.min, OP.add)
    return phi


def _build_body(nc, tc, qt, kk, vv, out):
    with (
        tc.tile_pool(name="io", bufs=4) as io,
        tc.tile_pool(name="ew", bufs=4) as ew,
        tc.tile_pool(name="misc", bufs=1) as misc,
        tc.tile_pool(name="small", bufs=3) as small,
        tc.tile_pool(name="outp", bufs=4) as outp,
    ):
        # ---------------- phase 1: A/b accumulation over S ----------------
        with tc.tile_pool(name="ps1", bufs=1, space="PSUM") as ps1:
            pacc = [
                ps1.tile([P, VA], F32, tag=f"pacc{g}", name=f"pacc{g}")
                for g in range(G)
            ]

            # HAM warm-up: a dense dummy matmul burst while the initial DMAs
            # prefill.  ~16 N=512 matmuls = ~5us of continuous PE activity
            # flips the clock gate to 8/8 (2.4 GHz); the real MM stream then
            # never idles long enough (>3.4us) to re-throttle.
            wz = misc.tile([P, 512], BF16, tag="warm", name="warm")
            nc.vector.memset(wz[:], 0.0)
            junk = ps1.tile([P, 512], F32, tag="junk", name="junk")
            for _ in range(16):
                nc.tensor.matmul(
                    junk[:], wz[:, 0:P], wz[:], start=True, stop=True
                )

            for m in range(N_MACRO):
                k_t = io.tile([P, MACRO * HD], BF16, tag="k_t")
                rows = slice(m * MACRO * P, (m + 1) * MACRO * P)
                nc.sync.dma_start(
                    k_t[:].rearrange("p (b c) -> p b c", b=MACRO),
                    kk[rows, :].rearrange("(b p) c -> p b c", p=P),
                )
                v_t = io.tile([P, MACRO * VR], BF16, tag="v_t")
                nc.scalar.dma_start(
                    v_t[:].rearrange("p (b c) -> p b c", b=MACRO),
                    vv[rows, :].rearrange("(b p) c -> p b c", p=P),
                )

                phi = _phi(nc, ew, k_t, MACRO * HD)

                first = m == 0
                last = m == N_MACRO - 1
                for b in range(MACRO):
                    for g in range(G):
                        nc.tensor.matmul(
                            pacc[g][:],
                            phi[:, b * HD + g * P : b * HD + (g + 1) * P],
                            v_t[:, b * VR + g * VA : b * VR + (g + 1) * VA],
                            start=(first and b == 0),
                            stop=(last and b == MACRO - 1),
                        )

            # ------------- phase 1.5: block-diag A, block-diag b ----------
            amat = []
            bmat = []
            for g in range(G):
                ag = misc.tile([P, P], BF16, tag=f"amat{g}", name=f"amat{g}")
                bg = misc.tile([P, 4], BF16, tag=f"bmat{g}", name=f"bmat{g}")
                nc.vector.memset(ag[:], 0.0)
                nc.vector.memset(bg[:], 0.0)
                for j in range(4):
                    r0 = 32 * j
                    nc.scalar.copy(
                        ag[r0 : r0 + 32, r0 : r0 + 32],
                        pacc[g][r0 : r0 + 32, r0 : r0 + 32],
                    )
                    nc.scalar.copy(
                        bg[r0 : r0 + 32, j : j + 1],
                        pacc[g][r0 : r0 + 32, P : P + 1],
                    )
                amat.append(ag)
                bmat.append(bg)

            # keep PE warm across the phase-1.5 transition
            for _ in range(6):
                nc.tensor.matmul(
                    junk[:], wz[:, 0:P], wz[:], start=True, stop=True
                )

        # ---------------- phase 2: queries ----------------
        with (
            tc.tile_pool(name="ps2n", bufs=5, space="PSUM") as ps2n,
            tc.tile_pool(name="ps2d", bufs=2, space="PSUM") as ps2d,
        ):
            for mq in range(N_QMACRO):
                c0 = mq * QMACRO * P
                phis = []
                for g in range(G):
                    qt_t = io.tile(
                        [P, QMACRO * P], BF16, tag=f"qt{g}", name=f"qt{g}"
                    )
                    nc.sync.dma_start(
                        qt_t[:], qt[g * P : (g + 1) * P, c0 : c0 + QMACRO * P]
                    )
                    phis.append(_phi(nc, ew, qt_t, QMACRO * P))

                # den PSUM for the whole macro: cols (sub, g, j)
                den_ps = ps2d.tile([P, QMACRO * G * 4], F32, tag="den_ps")
                numers = []
                nm = None
                for i in range(QMACRO):
                    if i % 2 == 0:
                        nm = ps2n.tile([P, 2 * HD], F32, tag="nm")
                        numers.append(nm)
                    for g in range(G):
                        w = phis[g][:, i * P : (i + 1) * P]
                        nc.tensor.matmul(
                            nm[:, (i % 2) * HD + g * P : (i % 2) * HD + (g + 1) * P],
                            w,
                            amat[g][:],
                            start=True,
                            stop=True,
                        )
                        nc.tensor.matmul(
                            den_ps[:, (i * G + g) * 4 : (i * G + g + 1) * 4],
                            w,
                            bmat[g][:],
                            start=True,
                            stop=True,
                        )

                rcp = small.tile([P, QMACRO * G * 4], F32, tag="rcp")
                nc.vector.reciprocal(rcp[:], den_ps[:])

                for pr in range(QMACRO // 2):
                    out_t = outp.tile([P, 2 * HD], BF16, tag="out_t")
                    rv = rcp[:, 2 * pr * G * 4 : (2 * pr + 2) * G * 4].rearrange(
                        "p (s g j) -> p s g j", s=2, g=G
                    )
                    nc.vector.tensor_tensor(
                        out_t[:].rearrange(
                            "p (s g j c) -> p s g j c", s=2, g=G, c=32
                        ),
                        numers[pr][:].rearrange(
                            "p (s g j c) -> p s g j c", s=2, g=G, c=32
                        ),
                        _bcast_last(rv, 32),
                        OP.mult,
                    )
                    r0 = c0 + 2 * pr * P
                    nc.scalar.dma_start(
                        out[r0 : r0 + 2 * P, :].rearrange("(s p) c -> p s c", p=P),
                        out_t[:].rearrange("p (s c) -> p s c", s=2),
                    )


_NC_CACHE = None


def build_nc():
    global _NC_CACHE
    if _NC_CACHE is not None:
        return _NC_CACHE
    nc = bacc.Bacc(
        "TRN2",
        target_bir_lowering=False,
        debug=False,
        enable_asserts=False,
        num_devices=N_BATCH,
    )
    qt = nc.dram_tensor("qt", [HD, L], BF16, kind="ExternalInput").ap()
    kk = nc.dram_tensor("kk", [S, HD], BF16, kind="ExternalInput").ap()
    vv = nc.dram_tensor("vv", [S, VR], BF16, kind="ExternalInput").ap()
    out = nc.dram_tensor("out", [L, HD], BF16, kind="ExternalOutput").ap()
    with tile.TileContext(nc) as tc:
        _build_body(nc, tc, qt, kk, vv, out)
    nc.compile()
    return nc


def make_in_maps(queries, keys, values):
    queries = np.asarray(queries, dtype=np.float32)
    keys = np.asarray(keys, dtype=np.float32)
    values = np.asarray(values, dtype=np.float32)
    bf = ml_dtypes.bfloat16
    in_maps = []
    for n in range(N_BATCH):
        v2 = values[n].reshape(S, HD)
        vva = np.ones((S, VR), dtype=bf)
        vva[:, 0:P] = v2[:, 0:P].astype(bf)
        vva[:, VA : VA + P] = v2[:, P : 2 * P].astype(bf)
        qt = np.ascontiguousarray(
            queries[n].transpose(1, 2, 0).reshape(HD, L).astype(bf)
        )  # [h*32+d, l]
        in_maps.append(
            {
                "qt": qt,
                "kk": np.ascontiguousarray(keys[n].reshape(S, HD).astype(bf)),
                "vv": vva,
            }
        )
    return in_maps


def run(queries, keys, values, trace=False, **kwargs):
    nc = build_nc()
    in_maps = make_in_maps(queries, keys, values)
    res = run_bass_kernel_spmd(
        nc, in_maps, core_ids=list(range(N_BATCH)), trace=trace, **kwargs
    )
    outs = [
        res.results[n]["out"].astype(np.float32).reshape(L, H, D)
        for n in range(N_BATCH)
    ]
    return np.stack(outs, axis=0), res


def kernel(queries, keys, values):
    out, _ = run(queries, keys, values, trace=False)
    return out


# revision 18
# speedup vs baseline: 1.0324x; 1.0289x over previous
"""Linear attention ("Transformers are RNNs") on 8 Trainium2 NeuronCores.

Problem: N=8, L=S=8192, H=8, D=Dv=32, f32.
    phi(x) = elu(x)+1
    A[d,v] = sum_s phi(K)[s,d] V[s,v]        (the /v_length ... *v_length cancels exactly)
    b[d]   = sum_s phi(K)[s,d]
    out[l,v] = (sum_d phi(Q)[l,d] A[d,v]) / (sum_d phi(Q)[l,d] b[d] + EPS)

Sharding: batch element n -> core n (fully independent, no collectives).

Device design (v4):
  - bf16 compute throughout (rel err ~4e-3, gate is 2e-2): inputs are
    cast to bf16 on the host, halving DMA traffic; PSUM accumulation and
    the denominator/normalization stay f32.
  - Q pre-transposed on host to [H*D, L]: contraction dim d on SBUF
    partitions, contiguous DMA, no on-device transposes.
  - phi(x) = min(exp(x), 1 + relu(x))  (exactly elu(x)+1):
    e = Exp(x) (ScalarE); t = (x max 0)+1 (VectorE dual-op tensor_scalar,
    4x mode); phi = min(e, t) (VectorE tt, 2x mode).
  - Phase 1 (64 s-subtiles of 128): per 4-head group g one bf16 matmul
        lhsT = phi(K)[:, g]  [s=128, (j,d)=128]
        rhs  = [V_g | ones]  [s=128, 129]
    accumulated in PSUM[128,129] per group.  Diagonal 32x32 j-blocks of
    cols 0:128 are A_h; col 128 is b_h.
  - Phase 1.5: assemble per group: block-diag A [128,128] bf16 and
    block-diag b columns [128,4] bf16.
  - Phase 2 (64 l-subtiles): per group two matmuls sharing the same
    stationary phiQ^T slice: numer (N=128, lands directly in output
    layout) and den (N=4, batched per 8-subtile macro in one PSUM bank so
    the eps-add + reciprocal amortize).  Normalize with one broadcast
    tensor_tensor multiply per 2 subtiles; bf16 out, host casts to f32.
"""

import sys

for _p in ("/opt/trn_rl_repo",):
    if _p not in sys.path:
        sys.path.insert(0, _p)

import ml_dtypes
import numpy as np

from concourse import bacc, bass, mybir, tile
from concourse.bass_utils import run_bass_kernel_spmd

# ---------------------------------------------------------------- constants
N_BATCH = 8
L = 8192
S = 8192
H = 8
D = 32
HD = H * D  # 256
P = 128
EPS = 1e-6

F32 = mybir.dt.float32
BF16 = mybir.dt.bfloat16
AF = mybir.ActivationFunctionType
OP = mybir.AluOpType

MACRO = 8  # 128-row s-subtiles per phase-1 macro tile
N_MACRO = S // (P * MACRO)  # 8
QMACRO = 4  # l-subtiles per phase-2 macro
N_QMACRO = L // (P * QMACRO)  # 16

G = 2  # head groups (4 heads each)
VA = P + 1  # 129: V group columns + ones column
VR = G * VA  # 258: host-side V row: [V_g0 | 1 | V_g1 | 1]


def _bcast_last(ap, n):
    """Append a stride-0 dim of size n to an AP (free-dim broadcast)."""
    ap = ap.unsqueeze(ap.ndim)
    return ap.broadcast_to(tuple(ap.shape[:-1]) + (n,))


def _phi(nc, pool, x, fd, pfx=""):
    """phi(x) = elu(x)+1 = min(exp(x), 1 + relu(x)); x is [P, fd] bf16 SBUF."""
    e = pool.tile([P, fd], BF16, tag=pfx + "phi_e", name=pfx + "phi_e")
    t = pool.tile([P, fd], BF16, tag=pfx + "phi_t", name=pfx + "phi_t")
    phi = pool.tile([P, fd], BF16, tag=pfx + "phi_o", name=pfx + "phi_o")
    nc.scalar.activation(e[:], x[:], AF.Exp)
    nc.vector.tensor_scalar(t[:], x[:], 0.0, 1.0, OP.max, OP.add)
    nc.vector.tensor_tensor(phi[:], e[:], t[:], OP.min)
    return phi


def _phi2(nc, pool, x, fd):
    """phi = (exp(x) min 1) + relu(x); exp and relu on ScalarE, one DVE
    scalar_tensor_tensor combines them (rebalances DVE -> ACT)."""
    e = pool.tile([P, fd], BF16, tag="phi_e")
    r = pool.tile([P, fd], BF16, tag="phi_r")
    phi = pool.tile([P, fd], BF16, tag="phi_o")
    nc.scalar.activation(e[:], x[:], AF.Exp)
    nc.scalar.activation(r[:], x[:], AF.Relu)
    nc.vector.scalar_tensor_tensor(phi[:], e[:], 1.0, r[:], OP.min, OP.add)
    return phi


def _build_body(nc, tc, qt, kk, vv, out):
    with (
        tc.tile_pool(name="io", bufs=4) as io,
        tc.tile_pool(name="ew", bufs=3) as ew,
        tc.tile_pool(name="ew2", bufs=6) as ew2,
        tc.tile_pool(name="misc", bufs=1) as misc,
        tc.tile_pool(name="small", bufs=3) as small,
        tc.tile_pool(name="outp", bufs=4) as outp,
    ):
        # ---------------- phase 1: A/b accumulation over S ----------------
        with tc.tile_pool(name="ps1", bufs=1, space="PSUM") as ps1:
            pacc = [
                ps1.tile([P, VA], F32, tag=f"pacc{g}", name=f"pacc{g}")
                for g in range(G)
            ]

            # HAM warm-up: a dense dummy matmul burst while the initial DMAs
            # prefill.  ~16 N=512 matmuls = ~5us of continuous PE activity
            # flips the clock gate to 8/8 (2.4 GHz); the real MM stream then
            # never idles long enough (>3.4us) to re-throttle.
            wz = misc.tile([P, 512], BF16, tag="warm", name="warm")
            nc.vector.memset(wz[:], 0.0)
            junk = ps1.tile([P, 512], F32, tag="junk", name="junk")
            for _ in range(16):
                nc.tensor.matmul(
                    junk[:], wz[:, 0:P], wz[:], start=True, stop=True
                )

            for m in range(N_MACRO):
                k_t = io.tile([P, MACRO * HD], BF16, tag="k_t")
                rows = slice(m * MACRO * P, (m + 1) * MACRO * P)
                nc.sync.dma_start(
                    k_t[:].rearrange("p (b c) -> p b c", b=MACRO),
                    kk[rows, :].rearrange("(b p) c -> p b c", p=P),
                )
                v_t = io.tile([P, MACRO * VR], BF16, tag="v_t")
                nc.scalar.dma_start(
                    v_t[:].rearrange("p (b c) -> p b c", b=MACRO),
                    vv[rows, :].rearrange("(b p) c -> p b c", p=P),
                )

                phi = _phi(nc, ew, k_t, MACRO * HD)

                first = m == 0
                last = m == N_MACRO - 1
                for b in range(MACRO):
                    for g in range(G):
                        nc.tensor.matmul(
                            pacc[g][:],
                            phi[:, b * HD + g * P : b * HD + (g + 1) * P],
                            v_t[:, b * VR + g * VA : b * VR + (g + 1) * VA],
                            start=(first and b == 0),
                            stop=(last and b == MACRO - 1),
                        )

            # ------------- phase 1.5: block-diag A, block-diag b ----------
            amat = []
            bmat = []
            for g in range(G):
                ag = misc.tile([P, P], BF16, tag=f"amat{g}", name=f"amat{g}")
                bg = misc.tile([P, 4], BF16, tag=f"bmat{g}", name=f"bmat{g}")
                nc.vector.memset(ag[:], 0.0)
                nc.vector.memset(bg[:], 0.0)
                for j in range(4):
                    r0 = 32 * j
                    nc.scalar.copy(
                        ag[r0 : r0 + 32, r0 : r0 + 32],
                        pacc[g][r0 : r0 + 32, r0 : r0 + 32],
                    )
                    nc.scalar.copy(
                        bg[r0 : r0 + 32, j : j + 1],
                        pacc[g][r0 : r0 + 32, P : P + 1],
                    )
                amat.append(ag)
                bmat.append(bg)

            # keep PE warm across the phase-1.5 transition
            for _ in range(6):
                nc.tensor.matmul(
                    junk[:], wz[:, 0:P], wz[:], start=True, stop=True
                )

        # ---------------- phase 2: queries ----------------
        with (
            tc.tile_pool(name="ps2n", bufs=5, space="PSUM") as ps2n,
            tc.tile_pool(name="ps2d", bufs=3, space="PSUM") as ps2d,
        ):
            for mq in range(N_QMACRO):
                c0 = mq * QMACRO * P
                phis = []
                for g in range(G):
                    qt_t = io.tile(
                        [P, QMACRO * P], BF16, tag=f"qt{g}", name=f"qt{g}"
                    )
                    nc.sync.dma_start(
                        qt_t[:], qt[g * P : (g + 1) * P, c0 : c0 + QMACRO * P]
                    )
                    phis.append(_phi(nc, ew2, qt_t, QMACRO * P, pfx="q"))

                # den PSUM for the whole macro: cols (sub, g, j)
                den_ps = ps2d.tile([P, QMACRO * G * 4], F32, tag="den_ps")
                numers = []
                nm = None
                for i in range(QMACRO):
                    if i % 2 == 0:
                        nm = ps2n.tile([P, 2 * HD], F32, tag="nm")
                        numers.append(nm)
                    for g in range(G):
                        w = phis[g][:, i * P : (i + 1) * P]
                        nc.tensor.matmul(
                            nm[:, (i % 2) * HD + g * P : (i % 2) * HD + (g + 1) * P],
                            w,
                            amat[g][:],
                            start=True,
                            stop=True,
                        )
                        nc.tensor.matmul(
                            den_ps[:, (i * G + g) * 4 : (i * G + g + 1) * 4],
                            w,
                            bmat[g][:],
                            start=True,
                            stop=True,
                        )

                rcp = small.tile([P, QMACRO * G * 4], F32, tag="rcp")
                nc.vector.reciprocal(rcp[:], den_ps[:])

                for pr in range(QMACRO // 2):
                    out_t = outp.tile([P, 2 * HD], BF16, tag="out_t")
                    rv = rcp[:, 2 * pr * G * 4 : (2 * pr + 2) * G * 4].rearrange(
                        "p (s g j) -> p s g j", s=2, g=G
                    )
                    nc.vector.tensor_tensor(
                        out_t[:].rearrange(
                            "p (s g j c) -> p s g j c", s=2, g=G, c=32
                        ),
                        numers[pr][:].rearrange(
                            "p (s g j c) -> p s g j c", s=2, g=G, c=32
                        ),
                        _bcast_last(rv, 32),
                        OP.mult,
                    )
                    r0 = c0 + 2 * pr * P
                    nc.scalar.dma_start(
                        out[r0 : r0 + 2 * P, :].rearrange("(s p) c -> p s c", p=P),
                        out_t[:].rearrange("p (s c) -> p s c", s=2),
                    )


_NC_CACHE = None


def build_nc():
    global _NC_CACHE
    if _NC_CACHE is not None:
        return _NC_CACHE
    nc = bacc.Bacc(
        "TRN2",
        target_bir_lowering=False,
        debug=False,
        enable_asserts=False,
        num_devices=N_BATCH,
    )
    qt = nc.dram_tensor("qt", [HD, L], BF16, kind="ExternalInput").ap()
    kk = nc.dram_tensor("kk", [S, HD], BF16, kind="ExternalInput").ap()
    vv = nc.dram_tensor("vv", [S, VR], BF16, kind="ExternalInput").ap()
    out = nc.dram_tensor("out", [L, HD], BF16, kind="ExternalOutput").ap()
    with tile.TileContext(nc) as tc:
        _build_body(nc, tc, qt, kk, vv, out)
    nc.compile()
    return nc


def make_in_maps(queries, keys, values):
    queries = np.asarray(queries, dtype=np.float32)
    keys = np.asarray(keys, dtype=np.float32)
    values = np.asarray(values, dtype=np.float32)
    bf = ml_dtypes.bfloat16
    in_maps = []
    for n in range(N_BATCH):
        v2 = values[n].reshape(S, HD)
        vva = np.ones((S, VR), dtype=bf)
        vva[:, 0:P] = v2[:, 0:P].astype(bf)
        vva[:, VA : VA + P] = v2[:, P : 2 * P].astype(bf)
        qt = np.ascontiguousarray(
            queries[n].transpose(1, 2, 0).reshape(HD, L).astype(bf)
        )  # [h*32+d, l]
        in_maps.append(
            {
                "qt": qt,
                "kk": np.ascontiguousarray(keys[n].reshape(S, HD).astype(bf)),
                "vv": vva,
            }
        )
    return in_maps


def run(queries, keys, values, trace=False, **kwargs):
    nc = build_nc()
    in_maps = make_in_maps(queries, keys, values)
    res = run_bass_kernel_spmd(
        nc, in_maps, core_ids=list(range(N_BATCH)), trace=trace, **kwargs
    )
    outs = [
        res.results[n]["out"].astype(np.float32).reshape(L, H, D)
        for n in range(N_BATCH)
    ]
    return np.stack(outs, axis=0), res


def kernel(queries, keys, values):
    out, _ = run(queries, keys, values, trace=False)
    return out


# revision 19
# speedup vs baseline: 1.0483x; 1.0154x over previous
"""Linear attention ("Transformers are RNNs") on 8 Trainium2 NeuronCores.

Problem: N=8, L=S=8192, H=8, D=Dv=32, f32.
    phi(x) = elu(x)+1
    A[d,v] = sum_s phi(K)[s,d] V[s,v]        (the /v_length ... *v_length cancels exactly)
    b[d]   = sum_s phi(K)[s,d]
    out[l,v] = (sum_d phi(Q)[l,d] A[d,v]) / (sum_d phi(Q)[l,d] b[d] + EPS)

Sharding: batch element n -> core n (fully independent, no collectives).

Device design (v4):
  - bf16 compute throughout (rel err ~4e-3, gate is 2e-2): inputs are
    cast to bf16 on the host, halving DMA traffic; PSUM accumulation and
    the denominator/normalization stay f32.
  - Q pre-transposed on host to [H*D, L]: contraction dim d on SBUF
    partitions, contiguous DMA, no on-device transposes.
  - phi(x) = min(exp(x), 1 + relu(x))  (exactly elu(x)+1):
    e = Exp(x) (ScalarE); t = (x max 0)+1 (VectorE dual-op tensor_scalar,
    4x mode); phi = min(e, t) (VectorE tt, 2x mode).
  - Phase 1 (64 s-subtiles of 128): per 4-head group g one bf16 matmul
        lhsT = phi(K)[:, g]  [s=128, (j,d)=128]
        rhs  = [V_g | ones]  [s=128, 129]
    accumulated in PSUM[128,129] per group.  Diagonal 32x32 j-blocks of
    cols 0:128 are A_h; col 128 is b_h.
  - Phase 1.5: assemble per group: block-diag A [128,128] bf16 and
    block-diag b columns [128,4] bf16.
  - Phase 2 (64 l-subtiles): per group two matmuls sharing the same
    stationary phiQ^T slice: numer (N=128, lands directly in output
    layout) and den (N=4, batched per 8-subtile macro in one PSUM bank so
    the eps-add + reciprocal amortize).  Normalize with one broadcast
    tensor_tensor multiply per 2 subtiles; bf16 out, host casts to f32.
"""

import sys

for _p in ("/opt/trn_rl_repo",):
    if _p not in sys.path:
        sys.path.insert(0, _p)

import ml_dtypes
import numpy as np

from concourse import bacc, bass, mybir, tile
from concourse.bass_utils import run_bass_kernel_spmd

# ---------------------------------------------------------------- constants
N_BATCH = 8
L = 8192
S = 8192
H = 8
D = 32
HD = H * D  # 256
P = 128
EPS = 1e-6

F32 = mybir.dt.float32
BF16 = mybir.dt.bfloat16
AF = mybir.ActivationFunctionType
OP = mybir.AluOpType

MACRO = 8  # 128-row s-subtiles per phase-1 macro tile
N_MACRO = S // (P * MACRO)  # 8
QMACRO = 4  # l-subtiles per phase-2 macro
N_QMACRO = L // (P * QMACRO)  # 16

G = 2  # head groups (4 heads each)
VA = P + 1  # 129: V group columns + ones column
VR = G * VA  # 258: host-side V row: [V_g0 | 1 | V_g1 | 1]


def _bcast_last(ap, n):
    """Append a stride-0 dim of size n to an AP (free-dim broadcast)."""
    ap = ap.unsqueeze(ap.ndim)
    return ap.broadcast_to(tuple(ap.shape[:-1]) + (n,))


def _phi(nc, pool, x, fd, pfx=""):
    """phi(x) = elu(x)+1 = min(exp(x), 1 + relu(x)); x is [P, fd] bf16 SBUF."""
    e = pool.tile([P, fd], BF16, tag=pfx + "phi_e", name=pfx + "phi_e")
    t = pool.tile([P, fd], BF16, tag=pfx + "phi_t", name=pfx + "phi_t")
    phi = pool.tile([P, fd], BF16, tag=pfx + "phi_o", name=pfx + "phi_o")
    nc.scalar.activation(e[:], x[:], AF.Exp)
    nc.vector.tensor_scalar(t[:], x[:], 0.0, 1.0, OP.max, OP.add)
    nc.vector.tensor_tensor(phi[:], e[:], t[:], OP.min)
    return phi


def _phi2(nc, pool, x, fd):
    """phi = (exp(x) min 1) + relu(x); exp and relu on ScalarE, one DVE
    scalar_tensor_tensor combines them (rebalances DVE -> ACT)."""
    e = pool.tile([P, fd], BF16, tag="phi_e")
    r = pool.tile([P, fd], BF16, tag="phi_r")
    phi = pool.tile([P, fd], BF16, tag="phi_o")
    nc.scalar.activation(e[:], x[:], AF.Exp)
    nc.scalar.activation(r[:], x[:], AF.Relu)
    nc.vector.scalar_tensor_tensor(phi[:], e[:], 1.0, r[:], OP.min, OP.add)
    return phi


def _build_body(nc, tc, qt, kk, vv, out):
    with (
        tc.tile_pool(name="io", bufs=4) as io,
        tc.tile_pool(name="ew", bufs=3) as ew,
        tc.tile_pool(name="ew2", bufs=10) as ew2,
        tc.tile_pool(name="misc", bufs=1) as misc,
        tc.tile_pool(name="small", bufs=3) as small,
        tc.tile_pool(name="outp", bufs=4) as outp,
    ):
        def _qprep(mq):
            c0 = mq * QMACRO * P
            ph = []
            for g in range(G):
                qt_t = io.tile([P, QMACRO * P], BF16, tag=f"qt{g}", name=f"qt{g}")
                nc.sync.dma_start(
                    qt_t[:], qt[g * P : (g + 1) * P, c0 : c0 + QMACRO * P]
                )
                ph.append(_phi(nc, ew2, qt_t, QMACRO * P, pfx="q"))
            return ph

        pre_phis = {}

        # ---------------- phase 1: A/b accumulation over S ----------------
        with tc.tile_pool(name="ps1", bufs=1, space="PSUM") as ps1:
            pacc = [
                ps1.tile([P, VA], F32, tag=f"pacc{g}", name=f"pacc{g}")
                for g in range(G)
            ]

            # HAM warm-up: a dense dummy matmul burst while the initial DMAs
            # prefill.  ~16 N=512 matmuls = ~5us of continuous PE activity
            # flips the clock gate to 8/8 (2.4 GHz); the real MM stream then
            # never idles long enough (>3.4us) to re-throttle.
            wz = misc.tile([P, 512], BF16, tag="warm", name="warm")
            nc.vector.memset(wz[:], 0.0)
            junk = ps1.tile([P, 512], F32, tag="junk", name="junk")
            for _ in range(16):
                nc.tensor.matmul(
                    junk[:], wz[:, 0:P], wz[:], start=True, stop=True
                )

            for m in range(N_MACRO):
                k_t = io.tile([P, MACRO * HD], BF16, tag="k_t")
                rows = slice(m * MACRO * P, (m + 1) * MACRO * P)
                nc.sync.dma_start(
                    k_t[:].rearrange("p (b c) -> p b c", b=MACRO),
                    kk[rows, :].rearrange("(b p) c -> p b c", p=P),
                )
                v_t = io.tile([P, MACRO * VR], BF16, tag="v_t")
                nc.scalar.dma_start(
                    v_t[:].rearrange("p (b c) -> p b c", b=MACRO),
                    vv[rows, :].rearrange("(b p) c -> p b c", p=P),
                )

                phi = _phi(nc, ew, k_t, MACRO * HD)

                first = m == 0
                last = m == N_MACRO - 1
                for b in range(MACRO):
                    for g in range(G):
                        nc.tensor.matmul(
                            pacc[g][:],
                            phi[:, b * HD + g * P : b * HD + (g + 1) * P],
                            v_t[:, b * VR + g * VA : b * VR + (g + 1) * VA],
                            start=(first and b == 0),
                            stop=(last and b == MACRO - 1),
                        )

                if m >= N_MACRO - 4:
                    pre_phis[m - (N_MACRO - 4)] = _qprep(m - (N_MACRO - 4))

            # ------------- phase 1.5: block-diag A, block-diag b ----------
            amat = []
            bmat = []
            for g in range(G):
                ag = misc.tile([P, P], BF16, tag=f"amat{g}", name=f"amat{g}")
                bg = misc.tile([P, 4], BF16, tag=f"bmat{g}", name=f"bmat{g}")
                nc.vector.memset(ag[:], 0.0)
                nc.vector.memset(bg[:], 0.0)
                for j in range(4):
                    r0 = 32 * j
                    nc.scalar.copy(
                        ag[r0 : r0 + 32, r0 : r0 + 32],
                        pacc[g][r0 : r0 + 32, r0 : r0 + 32],
                    )
                    nc.scalar.copy(
                        bg[r0 : r0 + 32, j : j + 1],
                        pacc[g][r0 : r0 + 32, P : P + 1],
                    )
                amat.append(ag)
                bmat.append(bg)

            # keep PE warm across the phase-1.5 transition
            for _ in range(6):
                nc.tensor.matmul(
                    junk[:], wz[:, 0:P], wz[:], start=True, stop=True
                )

        # ---------------- phase 2: queries ----------------
        with (
            tc.tile_pool(name="ps2n", bufs=5, space="PSUM") as ps2n,
            tc.tile_pool(name="ps2d", bufs=3, space="PSUM") as ps2d,
        ):
            for mq in range(N_QMACRO):
                c0 = mq * QMACRO * P
                phis = pre_phis.get(mq) or _qprep(mq)

                # den PSUM for the whole macro: cols (sub, g, j)
                den_ps = ps2d.tile([P, QMACRO * G * 4], F32, tag="den_ps")
                numers = []
                nm = None
                for i in range(QMACRO):
                    if i % 2 == 0:
                        nm = ps2n.tile([P, 2 * HD], F32, tag="nm")
                        numers.append(nm)
                    for g in range(G):
                        w = phis[g][:, i * P : (i + 1) * P]
                        nc.tensor.matmul(
                            nm[:, (i % 2) * HD + g * P : (i % 2) * HD + (g + 1) * P],
                            w,
                            amat[g][:],
                            start=True,
                            stop=True,
                        )
                        nc.tensor.matmul(
                            den_ps[:, (i * G + g) * 4 : (i * G + g + 1) * 4],
                            w,
                            bmat[g][:],
                            start=True,
                            stop=True,
                        )

                rcp = small.tile([P, QMACRO * G * 4], F32, tag="rcp")
                nc.vector.reciprocal(rcp[:], den_ps[:])

                for pr in range(QMACRO // 2):
                    out_t = outp.tile([P, 2 * HD], BF16, tag="out_t")
                    rv = rcp[:, 2 * pr * G * 4 : (2 * pr + 2) * G * 4].rearrange(
                        "p (s g j) -> p s g j", s=2, g=G
                    )
                    nc.vector.tensor_tensor(
                        out_t[:].rearrange(
                            "p (s g j c) -> p s g j c", s=2, g=G, c=32
                        ),
                        numers[pr][:].rearrange(
                            "p (s g j c) -> p s g j c", s=2, g=G, c=32
                        ),
                        _bcast_last(rv, 32),
                        OP.mult,
                    )
                    r0 = c0 + 2 * pr * P
                    nc.sync.dma_start(
                        out[r0 : r0 + 2 * P, :].rearrange("(s p) c -> p s c", p=P),
                        out_t[:].rearrange("p (s c) -> p s c", s=2),
                    )


_NC_CACHE = None


def build_nc():
    global _NC_CACHE
    if _NC_CACHE is not None:
        return _NC_CACHE
    nc = bacc.Bacc(
        "TRN2",
        target_bir_lowering=False,
        debug=False,
        enable_asserts=False,
        num_devices=N_BATCH,
    )
    qt = nc.dram_tensor("qt", [HD, L], BF16, kind="ExternalInput").ap()
    kk = nc.dram_tensor("kk", [S, HD], BF16, kind="ExternalInput").ap()
    vv = nc.dram_tensor("vv", [S, VR], BF16, kind="ExternalInput").ap()
    out = nc.dram_tensor("out", [L, HD], BF16, kind="ExternalOutput").ap()
    with tile.TileContext(nc) as tc:
        _build_body(nc, tc, qt, kk, vv, out)
    nc.compile()
    return nc


def make_in_maps(queries, keys, values):
    queries = np.asarray(queries, dtype=np.float32)
    keys = np.asarray(keys, dtype=np.float32)
    values = np.asarray(values, dtype=np.float32)
    bf = ml_dtypes.bfloat16
    in_maps = []
    for n in range(N_BATCH):
        v2 = values[n].reshape(S, HD)
        vva = np.ones((S, VR), dtype=bf)
        vva[:, 0:P] = v2[:, 0:P].astype(bf)
        vva[:, VA : VA + P] = v2[:, P : 2 * P].astype(bf)
        qt = np.ascontiguousarray(
            queries[n].transpose(1, 2, 0).reshape(HD, L).astype(bf)
        )  # [h*32+d, l]
        in_maps.append(
            {
                "qt": qt,
                "kk": np.ascontiguousarray(keys[n].reshape(S, HD).astype(bf)),
                "vv": vva,
            }
        )
    return in_maps


def run(queries, keys, values, trace=False, **kwargs):
    nc = build_nc()
    in_maps = make_in_maps(queries, keys, values)
    res = run_bass_kernel_spmd(
        nc, in_maps, core_ids=list(range(N_BATCH)), trace=trace, **kwargs
    )
    outs = [
        res.results[n]["out"].astype(np.float32).reshape(L, H, D)
        for n in range(N_BATCH)
    ]
    return np.stack(outs, axis=0), res


def kernel(queries, keys, values):
    out, _ = run(queries, keys, values, trace=False)
    return out
